# revision 1
# baseline (speedup 1.0000x reference)
"""Trainium2 Bass kernel for nn_BgeAttention (dense transformer block).

Sharding (8 NeuronCores): 2 batch groups x 4-way head/tensor parallel.
  core c: g = c//4 (batch), li = c%4 -> heads [4*li, 4*li+4)
  - QKV projections + attention for its 4 heads over the full 2048-token seq
  - partial o-proj (its 256 ctx dims) -> ReduceScatter(add) over the 4-core
    group, each core keeping tokens [512*li, 512*(li+1))
  - LN1 + FFN (bf16 weights) + LN2 on its 512-token slice
Matmuls run fp32r (TF32-like, full PE rate at free>=256); FFN matmuls bf16.
"""
import sys, os
sys.path.insert(0, '/opt/trn_rl_repo')
import numpy as np
import ml_dtypes
import concourse.bass as bass
import concourse.tile as tile
from concourse import bacc, mybir
from concourse.bass_utils import run_bass_kernel_spmd
from concourse.masks import make_identity

F32 = mybir.dt.float32
F32R = mybir.dt.float32r
BF16 = mybir.dt.bfloat16
AF = mybir.ActivationFunctionType
OP = mybir.AluOpType

S, D, HD, F = 2048, 1024, 64, 4096
GROUPS = [[0, 1, 2, 3], [4, 5, 6, 7]]
EPS = 1e-12

_CACHE = {}


def _bcast_ap(ap, p=128):
    return bass.AP(tensor=ap.tensor, offset=ap.offset, ap=[[0, p]] + list(ap.ap))


def _build():
    nc = bacc.Bacc("TRN2", target_bir_lowering=False, debug=False, num_devices=8)

    xg = nc.dram_tensor("xg", [S, D], F32, kind="ExternalInput").ap()
    wq = nc.dram_tensor("wq", [D, 256], F32R, kind="ExternalInput").ap()
    wk = nc.dram_tensor("wk", [D, 256], F32R, kind="ExternalInput").ap()
    wv = nc.dram_tensor("wv", [D, 256], F32R, kind="ExternalInput").ap()
    wo = nc.dram_tensor("wo", [256, D], F32R, kind="ExternalInput").ap()
    w1 = nc.dram_tensor("w1", [D, F], BF16, kind="ExternalInput").ap()
    w2 = nc.dram_tensor("w2", [F, D], BF16, kind="ExternalInput").ap()
    bq = nc.dram_tensor("bq", [256], F32, kind="ExternalInput").ap()
    bk = nc.dram_tensor("bk", [256], F32, kind="ExternalInput").ap()
    bv = nc.dram_tensor("bv", [256], F32, kind="ExternalInput").ap()
    bo = nc.dram_tensor("bo", [D], F32, kind="ExternalInput").ap()
    b1 = nc.dram_tensor("b1", [F], F32, kind="ExternalInput").ap()
    b2 = nc.dram_tensor("b2", [D], F32, kind="ExternalInput").ap()
    ln1g = nc.dram_tensor("ln1g", [D], F32, kind="ExternalInput").ap()
    ln1b = nc.dram_tensor("ln1b", [D], F32, kind="ExternalInput").ap()
    ln2g = nc.dram_tensor("ln2g", [D], F32, kind="ExternalInput").ap()
    ln2b = nc.dram_tensor("ln2b", [D], F32, kind="ExternalInput").ap()
    out = nc.dram_tensor("out", [512, D], F32, kind="ExternalOutput").ap()

    rs_in = nc.dram_tensor("rs_in", [S, D], F32)
    rs_out = nc.dram_tensor("rs_out", [512, D], F32)

    with tile.TileContext(nc) as tc:
        _emit(nc, tc, locals())
    nc.compile()
    return nc


def _emit(nc, tc, t):
    from contextlib import ExitStack
    from itertools import cycle
    PH = os.environ.get("BGE_KERNEL_PHASES", "full")
    xg, wq, wk, wv, wo, w1, w2 = t["xg"], t["wq"], t["wk"], t["wv"], t["wo"], t["w1"], t["w2"]
    bq, bk, bv, bo, b1, b2 = t["bq"], t["bk"], t["bv"], t["bo"], t["b1"], t["b2"]
    ln1g, ln1b, ln2g, ln2b = t["ln1g"], t["ln1b"], t["ln2g"], t["ln2b"]
    out, rs_in, rs_out = t["out"], t["rs_in"], t["rs_out"]

    with ExitStack() as top:
        const = top.enter_context(tc.tile_pool(name="const", bufs=1))
        stp = top.enter_context(tc.tile_pool(name="stp", bufs=2))

        ident = const.tile([128, 128], F32)
        make_identity(nc, ident[:])
        eps = const.tile([128, 1], F32)
        nc.vector.memset(eps[:], EPS)
        ones1f = const.tile([1, 64], F32)
        nc.vector.memset(ones1f[:], 1.0)
        ones1 = const.tile([1, 64], F32R)
        nc.vector.tensor_copy(ones1[:], ones1f[:])
        onesc = const.tile([128, 4, 1], F32)
        nc.vector.memset(onesc[:], 1.0)

        def bc_tile(src, n, name, pool=None):
            tl = (pool or const).tile([128, n], F32, name=name)
            nc.gpsimd.dma_start(out=tl[:], in_=_bcast_ap(src))
            return tl

        bv_b = bc_tile(bv, 256, "bv_b")
        lnp = top.enter_context(tc.tile_pool(name="lnp", bufs=1))
        A_t = [lnp.tile([128, D], F32, name=f"a{i}") for i in range(4)]

        def layernorm2p(dst, src, g_b, be_b):
            """LN with apply passes split across DVE (cols 0:640) and GpSimd (640:1024)."""
            stats = stp.tile([128, 2, 6], F32, name="stats")
            for sgi in range(2):
                nc.vector.bn_stats(out=stats[:, sgi, :], in_=src[:, sgi * 512:(sgi + 1) * 512])
            mv = stp.tile([128, 2], F32, name="mv")
            nc.vector.bn_aggr(out=mv[:], in_=stats[:])
            rstd = stp.tile([128, 1], F32, name="rstd")
            nc.scalar.activation(out=rstd[:], in_=mv[:, 1:2], func=AF.Sqrt,
                                 bias=eps[:], scale=1.0)
            nc.vector.reciprocal(out=rstd[:], in_=rstd[:])
            for eng, c0, c1 in ((nc.vector, 0, 640), (nc.gpsimd, 640, 1024)):
                eng.tensor_scalar(out=dst[:, c0:c1], in0=src[:, c0:c1],
                                  scalar1=mv[:, 0:1], scalar2=rstd[:],
                                  op0=OP.subtract, op1=OP.mult)
                eng.tensor_tensor(out=dst[:, c0:c1], in0=dst[:, c0:c1],
                                  in1=g_b[:, c0:c1], op=OP.mult)
                eng.tensor_tensor(out=dst[:, c0:c1], in0=dst[:, c0:c1],
                                  in1=be_b[:, c0:c1], op=OP.add)

        def layernorm(dst, src, g_b, be_b):
            stats = stp.tile([128, 2, 6], F32, name="stats")
            for sgi in range(2):
                nc.vector.bn_stats(out=stats[:, sgi, :], in_=src[:, sgi * 512:(sgi + 1) * 512])
            mv = stp.tile([128, 2], F32, name="mv")
            nc.vector.bn_aggr(out=mv[:], in_=stats[:])
            rstd = stp.tile([128, 1], F32, name="rstd")
            nc.scalar.activation(out=rstd[:], in_=mv[:, 1:2], func=AF.Sqrt,
                                 bias=eps[:], scale=1.0)
            nc.vector.reciprocal(out=rstd[:], in_=rstd[:])
            nc.vector.tensor_scalar(out=dst[:], in0=src[:], scalar1=mv[:, 0:1],
                                    scalar2=rstd[:], op0=OP.subtract, op1=OP.mult)
            nc.vector.tensor_mul(out=dst[:], in0=dst[:], in1=g_b[:])
            nc.vector.tensor_add(out=dst[:], in0=dst[:], in1=be_b[:])
        b1_sb = const.tile([128, 32], F32, name="b1_sb")
        nc.gpsimd.dma_start(out=b1_sb[:], in_=b1.rearrange("(a p) -> p a", p=128))
        bq_sb = const.tile([128, 2], F32, name="bq_sb")
        nc.gpsimd.dma_start(out=bq_sb[:], in_=bq.rearrange("(a p) -> p a", p=128))
        bk_sb = const.tile([128, 2], F32, name="bk_sb")
        nc.gpsimd.dma_start(out=bk_sb[:], in_=bk.rearrange("(a p) -> p a", p=128))

        with ExitStack() as ao_stack:
          octx = ao_stack.enter_context(tc.tile_pool(name="octx", bufs=1))
          Ctx = [octx.tile([128, S], F32R, name=f"ctx{i}") for i in range(2)]
          wo_t = octx.tile([128, 2, D], F32R, name="wo_t")
          for dc2 in range(2):
              (nc.gpsimd, nc.sync)[dc2].dma_start(
                  out=wo_t[:, dc2, :], in_=wo[dc2 * 128:(dc2 + 1) * 128, :])
          with ExitStack() as att_stack:
              attp = att_stack.enter_context(tc.tile_pool(name="attp", bufs=1))
              Qt = [attp.tile([128, S], F32R, name=f"qt{i}") for i in range(2)]
              Kt = [attp.tile([128, S], F32R, name=f"kt{i}") for i in range(2)]
              Vaug = [attp.tile([128, 4, 65], F32R, name=f"va{kc}") for kc in range(16)]

              # ------------- Phase P: transpose x + QKV projections -------------
              with ExitStack() as ph:
                  psA = ph.enter_context(tc.tile_pool(name="psP", bufs=4, space="PSUM"))
                  xap = ph.enter_context(tc.tile_pool(name="xap", bufs=3))
                  xtp = ph.enter_context(tc.tile_pool(name="xtp", bufs=1))
                  wp = ph.enter_context(tc.tile_pool(name="wp", bufs=1))
                  wq_t = wp.tile([128, 8, 256], F32R, name="wq_t")
                  wk_t = wp.tile([128, 8, 256], F32R, name="wk_t")
                  wv_t = wp.tile([128, 8, 256], F32R, name="wv_t")
                  _xengs = cycle((nc.sync, nc.gpsimd, nc.scalar))

                  def load_xa(ts):
                      xa = xap.tile([128, 4, D], F32, name="xa")
                      for tc4 in range(4):
                          nstrip = 4 if ts == 0 else 2
                          w = 1024 // nstrip
                          for hh in range(nstrip):
                              next(_xengs).dma_start(
                                  out=xa[:, tc4, hh * w:(hh + 1) * w],
                                  in_=xg[ts * 512 + tc4 * 128:
                                         ts * 512 + (tc4 + 1) * 128,
                                         hh * w:(hh + 1) * w])
                      return xa

                  xas = {0: load_xa(0)}
                  _wengs = cycle((nc.gpsimd, nc.sync, nc.scalar))
                  for wi, (_wt, _w) in enumerate(((wk_t, wk), (wq_t, wq), (wv_t, wv))):
                      for g4 in range(2):
                          next(_wengs).dma_start(
                              out=_wt[:, g4 * 4:(g4 + 1) * 4, :],
                              in_=_w[g4 * 512:(g4 + 1) * 512, :].rearrange(
                                  "(a p) f -> p a f", p=128))
                      xas[wi + 1] = load_xa(wi + 1)

                  for ts in range(4):
                      xa = xas[ts]
                      Xts = [xtp.tile([128, 512], F32R, name=f"xt{dc}") for dc in range(8)]
                      for tc4 in range(4):
                          for dc in range(8):
                              pt = psA.tile([128, 128], F32, name="ps")
                              nc.tensor.transpose(pt[:], xa[:, tc4, dc * 128:(dc + 1) * 128], ident[:])
                              nc.vector.tensor_copy(Xts[dc][:, tc4 * 128:(tc4 + 1) * 128], pt[:])
                      for w_t, b_sb, Dst in ((wk_t, bk_sb, Kt), (wq_t, bq_sb, Qt)):
                          for oc in range(2):
                              pk = psA.tile([128, 512], F32, name="ps")
                              for dc in range(8):
                                  nc.tensor.matmul(pk[:], w_t[:, dc, oc * 128:(oc + 1) * 128],
                                                   Xts[dc][:], start=(dc == 0), stop=(dc == 7))
                              nc.vector.tensor_scalar_add(
                                  out=Dst[oc][:, ts * 512:(ts + 1) * 512], in0=pk[:],
                                  scalar1=b_sb[:, oc:oc + 1])
                      for tc4 in range(4):
                          kc = ts * 4 + tc4
                          pv = psA.tile([128, 256], F32, name="ps")
                          for dc in range(8):
                              nc.tensor.matmul(pv[:], Xts[dc][:, tc4 * 128:(tc4 + 1) * 128],
                                               wv_t[:, dc, :], start=(dc == 0), stop=(dc == 7))
                          nc.vector.tensor_tensor(
                              out=Vaug[kc][:, :, 0:64],
                              in0=pv[:].rearrange("p (h d) -> p h d", h=4),
                              in1=bv_b[:].rearrange("p (h d) -> p h d", h=4),
                              op=OP.add)
                          nc.vector.tensor_copy(Vaug[kc][:, :, 64:65], onesc[:])

              # ---- Phase A+O: attention interleaved with o-proj + chunked RS ----
              with ExitStack() as ph:
                if PH in ("pa", "pao", "paor", "paof", "full"):
                  expp = ph.enter_context(tc.tile_pool(name="expp", bufs=2))
                  rzp = ph.enter_context(tc.tile_pool(name="rzp", bufs=2))
                  stgp = ph.enter_context(tc.tile_pool(name="stgp", bufs=1))
                  scP = ph.enter_context(tc.tile_pool(name="scP", bufs=2, space="PSUM"))
                  psO = ph.enter_context(tc.tile_pool(name="psO", bufs=1, space="PSUM"))
                  psB = ph.enter_context(tc.tile_pool(name="psB", bufs=2, space="PSUM"))
                  psC = ph.enter_context(tc.tile_pool(name="psC", bufs=1, space="PSUM"))
                  lnc1 = ph.enter_context(tc.tile_pool(name="lnc1", bufs=1))
                  rawp = ph.enter_context(tc.tile_pool(name="rawp", bufs=2))
                  do_o = PH in ("pao", "paor", "paof", "full")
                  do_rs = PH in ("paor", "full")
                  bo_b = bc_tile(bo, D, "bo_b", lnc1)
                  ln1g_b = bc_tile(ln1g, D, "ln1g_b", lnc1)
                  ln1b_b = bc_tile(ln1b, D, "ln1b_b", lnc1)
                  for qb in range(4):
                      for hp in range(2):
                          avs = [psB.tile([65, 512], F32, name="av") for i in range(2)]
                          for kp in range(8):
                              for i in range(2):
                                  sc = scP.tile([128, 1024], F32, name="sc2")
                                  for half in range(2):
                                      kc = 2 * kp + half
                                      nc.tensor.matmul(
                                          sc[:, half * 512:(half + 1) * 512],
                                          Kt[hp][i * 64:(i + 1) * 64, kc * 128:(kc + 1) * 128],
                                          Qt[hp][i * 64:(i + 1) * 64, qb * 512:(qb + 1) * 512],
                                          start=True, stop=True)
                                  e = expp.tile([128, 1024], F32R, name=f"e{i}")
                                  nc.scalar.activation(e[:], sc[:], AF.Exp)
                                  for half in range(2):
                                      kc = 2 * kp + half
                                      nc.tensor.matmul(
                                          avs[i][:], Vaug[kc][:, 2 * hp + i, :],
                                          e[:, half * 512:(half + 1) * 512],
                                          start=(kc == 0), stop=(kc == 15))
                          for i in range(2):
                              rz = rzp.tile([1, 512], F32R, name="rz")
                              with nc.allow_low_precision(reason="f32r is full width"):
                                  nc.vector.reciprocal(rz[:], avs[i][64:65, :])
                              bcp = psC.tile([64, 512], F32, name="bcp")
                              nc.tensor.matmul(bcp[:], ones1[:], rz[:], start=True, stop=True)
                              rzs = rzp.tile([64, 512], F32, name="rzs")
                              nc.vector.tensor_copy(rzs[:], bcp[:])
                              nc.vector.tensor_mul(
                                  out=Ctx[hp][i * 64:(i + 1) * 64, qb * 512:(qb + 1) * 512],
                                  in0=avs[i][0:64, :], in1=rzs[:])
                      if PH in ("pao", "paor", "paof", "full"):
                          sA = stgp.tile([128, 4, D], F32, name="sA")
                          for q4 in range(4):
                              tc16 = qb * 4 + q4
                              for oh in range(2):
                                  po = psO.tile([128, 512], F32, name="po")
                                  for dc2 in range(2):
                                      nc.tensor.matmul(
                                          po[:], Ctx[dc2][:, tc16 * 128:(tc16 + 1) * 128],
                                          wo_t[:, dc2, oh * 512:(oh + 1) * 512],
                                          start=(dc2 == 0), stop=(dc2 == 1))
                                  nc.vector.tensor_copy(sA[:, q4, oh * 512:(oh + 1) * 512], po[:])
                          for q4 in range(4):
                              (nc.sync, nc.gpsimd, nc.scalar, nc.sync)[q4].dma_start(
                                  out=rs_in[(qb * 4 + q4) * 128:(qb * 4 + q4 + 1) * 128, :],
                                  in_=sA[:, q4, :])
                      if PH in ("paor", "full"):
                          nc.gpsimd.collective_compute(
                              "ReduceScatter", OP.add,
                              ins=[rs_in[qb * 512:(qb + 1) * 512, :]],
                              outs=[rs_out[qb * 128:(qb + 1) * 128, :]],
                              replica_groups=GROUPS)
                      if do_o and PH in ("paor", "paof", "full"):
                          # LN1 for this chunk as soon as its RS lands
                          raw = rawp.tile([128, D], F32, name="raw")
                          for hh in range(2):
                              (nc.sync, nc.gpsimd)[hh].dma_start(
                                  out=raw[:, hh * 512:(hh + 1) * 512],
                                  in_=rs_out[qb * 128:(qb + 1) * 128,
                                             hh * 512:(hh + 1) * 512])
                          nc.vector.tensor_add(out=raw[:], in0=raw[:], in1=bo_b[:])
                          layernorm(A_t[qb], raw, ln1g_b, ln1b_b)

        # ------------- Phase F: LN1 + FFN + LN2 -------------
        if PH not in ("full", "paof"):
            return
        with ExitStack() as ph:
            lnc2 = ph.enter_context(tc.tile_pool(name="lnc2", bufs=1))
            b2_b = bc_tile(b2, D, "b2_b", lnc2)
            ln2g_b = bc_tile(ln2g, D, "ln2g_b", lnc2)
            ln2b_b = bc_tile(ln2b, D, "ln2b_b", lnc2)
            sbA = ph.enter_context(tc.tile_pool(name="sbA", bufs=1))
            w1p = ph.enter_context(tc.tile_pool(name="w1p", bufs=2))
            w2p = ph.enter_context(tc.tile_pool(name="w2p", bufs=2))
            hp_ = ph.enter_context(tc.tile_pool(name="hp", bufs=2))
            fmisc = ph.enter_context(tc.tile_pool(name="fmisc", bufs=2))
            psA = ph.enter_context(tc.tile_pool(name="psF", bufs=4, space="PSUM"))
            psD = ph.enter_context(tc.tile_pool(name="psD", bufs=4, space="PSUM"))

            At = [sbA.tile([128, 512], BF16, name=f"at{dc}") for dc in range(8)]
            for tc4 in range(4):
                for dc in range(8):
                    pt = psA.tile([128, 128], F32, name="ps")
                    nc.tensor.transpose(pt[:], A_t[tc4][:, dc * 128:(dc + 1) * 128], ident[:])
                    nc.scalar.copy(At[dc][:, tc4 * 128:(tc4 + 1) * 128], pt[:])

            ffn_acc = [sbA.tile([128, D], F32, name=f"fa{i}") for i in range(4)]
            oall = sbA.tile([128, 4, D], F32, name="oall")
            for tc4 in range(4):
                nc.vector.tensor_add(out=ffn_acc[tc4][:], in0=A_t[tc4][:], in1=b2_b[:])
            for fg in range(4):
                w1t = w1p.tile([128, 8, 1024], BF16, name="w1t")
                w2t = w2p.tile([128, 8, D], BF16, name="w2t")
                _fengs = cycle((nc.gpsimd, nc.sync, nc.scalar))
                for g4 in range(4):
                    next(_fengs).dma_start(
                        out=w1t[:, g4 * 2:(g4 + 1) * 2, :],
                        in_=w1[g4 * 256:(g4 + 1) * 256, fg * 1024:(fg + 1) * 1024].rearrange(
                            "(a p) f -> p a f", p=128))
                for g4 in range(4):
                    next(_fengs).dma_start(
                        out=w2t[:, g4 * 2:(g4 + 1) * 2, :],
                        in_=w2[fg * 1024 + g4 * 256: fg * 1024 + (g4 + 1) * 256, :].rearrange(
                            "(a p) f -> p a f", p=128))
                hts = [hp_.tile([128, 512], BF16, name=f"h{fc}") for fc in range(8)]
                for fc8 in range(8):
                    phm = psA.tile([128, 512], F32, name="ps")
                    for dc in range(8):
                        nc.tensor.matmul(phm[:], w1t[:, dc, fc8 * 128:(fc8 + 1) * 128],
                                         At[dc][:], start=(dc == 0), stop=(dc == 7))
                    fci = fg * 8 + fc8
                    tmp = fmisc.tile([128, 512], F32, name="tmp")
                    nc.vector.tensor_scalar(out=tmp[:], in0=phm[:],
                                            scalar1=b1_sb[:, fci:fci + 1], scalar2=0.0,
                                            op0=OP.add, op1=OP.max)
                    nc.scalar.activation(hts[fc8][:], tmp[:], AF.Gelu)
                if fg < 3:
                    for oh in range(2):
                        paccs = [psD.tile([128, 512], F32, name="pac") for i in range(4)]
                        for fc8 in range(8):
                            for tc4 in range(4):
                                nc.tensor.matmul(paccs[tc4][:],
                                                 hts[fc8][:, tc4 * 128:(tc4 + 1) * 128],
                                                 w2t[:, fc8, oh * 512:(oh + 1) * 512],
                                                 start=(fc8 == 0), stop=(fc8 == 7))
                        for tc4 in range(4):
                            dst = ffn_acc[tc4][:, oh * 512:(oh + 1) * 512]
                            nc.vector.tensor_add(out=dst, in0=dst, in1=paccs[tc4][:])
                else:
                    # last group tc4-major: LN2(tc4) overlaps fc2 of tc4+1
                    for tc4 in range(4):
                        paccs = [psD.tile([128, 512], F32, name="pac") for i in range(2)]
                        for oh in range(2):
                            for fc8 in range(8):
                                nc.tensor.matmul(paccs[oh][:],
                                                 hts[fc8][:, tc4 * 128:(tc4 + 1) * 128],
                                                 w2t[:, fc8, oh * 512:(oh + 1) * 512],
                                                 start=(fc8 == 0), stop=(fc8 == 7))
                        acc = ffn_acc[tc4]
                        for oh in range(2):
                            dst = acc[:, oh * 512:(oh + 1) * 512]
                            nc.vector.tensor_add(out=dst, in0=dst, in1=paccs[oh][:])
                        layernorm2p(oall[:, tc4, :], acc, ln2g_b, ln2b_b)
                        for s4 in range(4):
                            (nc.sync, nc.gpsimd, nc.scalar, nc.sync)[s4].dma_start(
                                out=out[tc4 * 128:(tc4 + 1) * 128,
                                        s4 * 256:(s4 + 1) * 256],
                                in_=oall[:, tc4, s4 * 256:(s4 + 1) * 256])


def _get_nc():
    if "nc" not in _CACHE:
        _CACHE["nc"] = _build()
    return _CACHE["nc"]


def _in_maps(inputs):
    x = np.asarray(inputs["x"], dtype=np.float32)
    maps = []
    for c in range(8):
        g, li = c // 4, c % 4
        cs = slice(256 * li, 256 * (li + 1))
        m = {
            "xg": np.ascontiguousarray(x[g]),
            "wq": np.ascontiguousarray(np.asarray(inputs["Wq"], np.float32)[:, cs]) / 8.0,
            "wk": np.ascontiguousarray(np.asarray(inputs["Wk"], np.float32)[:, cs]),
            "wv": np.ascontiguousarray(np.asarray(inputs["Wv"], np.float32)[:, cs]),
            "wo": np.ascontiguousarray(np.asarray(inputs["Wo"], np.float32)[cs, :]),
            "w1": np.asarray(inputs["W1"], np.float32).astype(ml_dtypes.bfloat16),
            "w2": np.asarray(inputs["W2"], np.float32).astype(ml_dtypes.bfloat16),
            "bq": np.ascontiguousarray(np.asarray(inputs["bq"], np.float32)[cs]) / 8.0,
            "bk": np.ascontiguousarray(np.asarray(inputs["bk"], np.float32)[cs]),
            "bv": np.ascontiguousarray(np.asarray(inputs["bv"], np.float32)[cs]),
            "bo": np.asarray(inputs["bo"], np.float32),
            "b1": np.asarray(inputs["b1"], np.float32),
            "b2": np.asarray(inputs["b2"], np.float32),
            "ln1g": np.asarray(inputs["ln1_g"], np.float32),
            "ln1b": np.asarray(inputs["ln1_b"], np.float32),
            "ln2g": np.asarray(inputs["ln2_g"], np.float32),
            "ln2b": np.asarray(inputs["ln2_b"], np.float32),
        }
        maps.append(m)
    return maps


def run(inputs, trace=False):
    nc = _get_nc()
    res = run_bass_kernel_spmd(nc, _in_maps(inputs), list(range(8)), trace=trace)
    B = 2
    full = np.empty((B, S, D), np.float32)
    for c in range(8):
        g, li = c // 4, c % 4
        o = res.results[c]["out"]
        for j in range(4):
            full[g, j * 512 + li * 128: j * 512 + (li + 1) * 128, :] = \
                o[j * 128:(j + 1) * 128]
    return full, res


def kernel(**inputs):
    return run(inputs)[0]



# revision 3
# speedup vs baseline: 100.0379x; 100.0379x over previous
"""Trainium2 Bass kernel for nn_BgeAttention (dense transformer block), v2.

Sharding (8 NeuronCores): 2 batch groups x 4-way head/tensor parallel.
  core c: g = c//4 (batch), li = c%4 -> heads [4*li, 4*li+4), ctx dims
  [256*li, +256), output tokens [512*li, +512) (contiguous).
  - QKV projections + attention for its 4 heads over the full 2048-token seq
    (all bf16 on the PE; f32 PSUM accumulation)
  - partial o-proj (its 256 ctx dims) -> chunked bf16 ReduceScatter(add)
    over the 4-core group (half the f32 traffic), each core keeping tokens
    [qb*512 + 128*li) per chunk; LN1 + FFN on its 512 tokens.
  - FFN fc1/fc2 run fp8(e4m3) DoubleRow matmuls (2 k-tiles fused per
    instruction); weights are pre-scaled x64 on the host to stay in the
    e4m3 normal range, and the x64 is folded out exactly via LN2's scale
    invariance (fc2 path) and the activation's scale input (fc1 path).
Unrolled BGE_KERNEL_REPS times for slope-based device timing.
"""
import sys, os
sys.path.insert(0, '/opt/trn_rl_repo')
import numpy as np
import ml_dtypes
import concourse.bass as bass
import concourse.tile as tile
from concourse import bacc, mybir
from concourse.bass_utils import run_bass_kernel_spmd
from concourse.masks import make_identity

F32 = mybir.dt.float32
F32R = mybir.dt.float32r
BF16 = mybir.dt.bfloat16
F8 = mybir.dt.float8e4
AF = mybir.ActivationFunctionType
OP = mybir.AluOpType
DR = mybir.MatmulPerfMode.DoubleRow

S, D, HD, F = 2048, 1024, 64, 4096
GROUPS = [[0, 1, 2, 3], [4, 5, 6, 7]]
EPS = 1e-12
WSC = 64.0  # host-side fp8 weight pre-scale
FC1_FP8 = os.environ.get("BGE_FC1", "bf16") == "fp8"
FC2_FP8 = os.environ.get("BGE_FC2", "bf16") == "fp8"
W2DT_NP = None  # set in _in_maps

_CACHE = {}


def _bcast_ap(ap, p=128):
    return bass.AP(tensor=ap.tensor, offset=ap.offset, ap=[[0, p]] + list(ap.ap))


def _build():
    nc = bacc.Bacc("TRN2", target_bir_lowering=False, debug=False, num_devices=8)

    xg = nc.dram_tensor("xg", [S, D], BF16, kind="ExternalInput").ap()
    wq = nc.dram_tensor("wq", [D, 256], BF16, kind="ExternalInput").ap()
    wk = nc.dram_tensor("wk", [D, 256], BF16, kind="ExternalInput").ap()
    wv = nc.dram_tensor("wv", [D, 256], BF16, kind="ExternalInput").ap()
    wo = nc.dram_tensor("wo", [256, D], BF16, kind="ExternalInput").ap()
    w1 = nc.dram_tensor("w1", [D, F], F8 if FC1_FP8 else BF16, kind="ExternalInput").ap()
    w2 = nc.dram_tensor("w2", [F, D], F8 if FC2_FP8 else BF16, kind="ExternalInput").ap()
    bq = nc.dram_tensor("bq", [256], F32, kind="ExternalInput").ap()
    bk = nc.dram_tensor("bk", [256], F32, kind="ExternalInput").ap()
    bv = nc.dram_tensor("bv", [256], F32, kind="ExternalInput").ap()
    bo = nc.dram_tensor("bo", [D], F32, kind="ExternalInput").ap()
    b1 = nc.dram_tensor("b1", [F], F32, kind="ExternalInput").ap()
    b2 = nc.dram_tensor("b2", [D], F32, kind="ExternalInput").ap()
    ln1g = nc.dram_tensor("ln1g", [D], F32, kind="ExternalInput").ap()
    ln1b = nc.dram_tensor("ln1b", [D], F32, kind="ExternalInput").ap()
    ln2g = nc.dram_tensor("ln2g", [D], F32, kind="ExternalInput").ap()
    ln2b = nc.dram_tensor("ln2b", [D], F32, kind="ExternalInput").ap()
    out = nc.dram_tensor("out", [512, D], F32, kind="ExternalOutput").ap()

    rs_in = nc.dram_tensor("rs_in", [S, D], BF16)
    rs_out = nc.dram_tensor("rs_out", [512, D], BF16)

    reps = int(os.environ.get("BGE_KERNEL_REPS", "1"))
    t = dict(locals())
    with tile.TileContext(nc) as tc:
        from contextlib import ExitStack
        with ExitStack() as top:
            const = top.enter_context(tc.tile_pool(name="const", bufs=1))
            _emit_consts(nc, tc, const, t)
            for _rep in range(reps):
                _emit(nc, tc, t)
    nc.compile()
    return nc


def _emit_consts(nc, tc, const, t):
    bq, bk, bv, bo, b1, b2 = t["bq"], t["bk"], t["bv"], t["bo"], t["b1"], t["b2"]
    ln1g, ln1b, ln2g, ln2b = t["ln1g"], t["ln1b"], t["ln2g"], t["ln2b"]

    ident = const.tile([128, 128], BF16)
    make_identity(nc, ident[:])
    eps = const.tile([128, 1], F32)
    nc.vector.memset(eps[:], EPS)
    ones1f = const.tile([1, 64], F32)
    nc.vector.memset(ones1f[:], 1.0)
    ones1 = const.tile([1, 64], F32R)
    nc.vector.tensor_copy(ones1[:], ones1f[:])
    onesc = const.tile([128, 4, 1], F32)
    nc.vector.memset(onesc[:], 1.0)

    def bc_tile(src, n, name):
        tl = const.tile([128, n], F32, name=name)
        nc.gpsimd.dma_start(out=tl[:], in_=_bcast_ap(src))
        return tl

    t["c_ident"], t["c_eps"], t["c_ones1"], t["c_onesc"] = ident, eps, ones1, onesc
    t["c_bv_b"] = bc_tile(bv, 256, "bv_b")
    t["c_bo_b"] = bc_tile(bo, D, "bo_b")
    t["c_ln1g_b"] = bc_tile(ln1g, D, "ln1g_b")
    t["c_ln1b_b"] = bc_tile(ln1b, D, "ln1b_b")
    t["c_b2_b"] = bc_tile(b2, D, "b2_b")
    t["c_ln2g_b"] = bc_tile(ln2g, D, "ln2g_b")
    t["c_ln2b_b"] = bc_tile(ln2b, D, "ln2b_b")
    b1_sb = const.tile([128, 32], F32, name="b1_sb")
    nc.gpsimd.dma_start(out=b1_sb[:], in_=b1.rearrange("(a p) -> p a", p=128))
    bq_sb = const.tile([128, 2], F32, name="bq_sb")
    nc.gpsimd.dma_start(out=bq_sb[:], in_=bq.rearrange("(a p) -> p a", p=128))
    bk_sb = const.tile([128, 2], F32, name="bk_sb")
    nc.gpsimd.dma_start(out=bk_sb[:], in_=bk.rearrange("(a p) -> p a", p=128))
    t["c_b1_sb"], t["c_bq_sb"], t["c_bk_sb"] = b1_sb, bq_sb, bk_sb


def _emit(nc, tc, t):
    from contextlib import ExitStack
    from itertools import cycle
    xg, wq, wk, wv, wo, w1, w2 = t["xg"], t["wq"], t["wk"], t["wv"], t["wo"], t["w1"], t["w2"]
    out, rs_in, rs_out = t["out"], t["rs_in"], t["rs_out"]
    ident, eps, ones1, onesc = t["c_ident"], t["c_eps"], t["c_ones1"], t["c_onesc"]
    bv_b, bo_b, ln1g_b, ln1b_b = t["c_bv_b"], t["c_bo_b"], t["c_ln1g_b"], t["c_ln1b_b"]
    b2_b, ln2g_b, ln2b_b = t["c_b2_b"], t["c_ln2g_b"], t["c_ln2b_b"]
    b1_sb, bq_sb, bk_sb = t["c_b1_sb"], t["c_bq_sb"], t["c_bk_sb"]

    with ExitStack() as top:
        stp = top.enter_context(tc.tile_pool(name="stp", bufs=2))
        lnp = top.enter_context(tc.tile_pool(name="lnp", bufs=1))
        A_t = [lnp.tile([128, D], BF16, name=f"a{i}") for i in range(4)]
        w1p = top.enter_context(tc.tile_pool(name="w1p", bufs=2))
        w2p = top.enter_context(tc.tile_pool(name="w2p", bufs=2))

        def layernorm(dst, src, g_b, be_b, split=False):
            stats = stp.tile([128, 2, 6], F32, name="stats")
            for sgi in range(2):
                nc.vector.bn_stats(out=stats[:, sgi, :], in_=src[:, sgi * 512:(sgi + 1) * 512])
            mv = stp.tile([128, 2], F32, name="mv")
            nc.vector.bn_aggr(out=mv[:], in_=stats[:])
            rstd = stp.tile([128, 1], F32, name="rstd")
            nc.scalar.activation(out=rstd[:], in_=mv[:, 1:2], func=AF.Sqrt,
                                 bias=eps[:], scale=1.0)
            nc.vector.reciprocal(out=rstd[:], in_=rstd[:])
            engs = ((nc.vector, 0, 640), (nc.gpsimd, 640, 1024)) if split \
                else ((nc.vector, 0, 1024),)
            for eng, c0, c1 in engs:
                eng.tensor_scalar(out=dst[:, c0:c1], in0=src[:, c0:c1],
                                  scalar1=mv[:, 0:1], scalar2=rstd[:],
                                  op0=OP.subtract, op1=OP.mult)
                eng.tensor_tensor(out=dst[:, c0:c1], in0=dst[:, c0:c1],
                                  in1=g_b[:, c0:c1], op=OP.mult)
                eng.tensor_tensor(out=dst[:, c0:c1], in0=dst[:, c0:c1],
                                  in1=be_b[:, c0:c1], op=OP.add)

        _fengs = cycle((nc.gpsimd, nc.sync, nc.scalar))

        def load_fw(wp, wsrc, fg, which):
            dt = F8 if ((which == 1 and FC1_FP8) or (which == 2 and FC2_FP8)) else BF16
            wt = wp.tile([128, 8, 1024], dt, name=f"w{which}t")
            if which == 1:
                for g4 in range(4):
                    next(_fengs).dma_start(
                        out=wt[:, g4 * 2:(g4 + 1) * 2, :],
                        in_=wsrc[g4 * 256:(g4 + 1) * 256,
                                 fg * 1024:(fg + 1) * 1024].rearrange(
                            "(a p) f -> p a f", p=128))
            else:
                for g4 in range(4):
                    next(_fengs).dma_start(
                        out=wt[:, g4 * 2:(g4 + 1) * 2, :],
                        in_=wsrc[fg * 1024 + g4 * 256: fg * 1024 + (g4 + 1) * 256,
                                 :].rearrange("(a p) f -> p a f", p=128))
            return wt

        with ExitStack() as att_stack:
            attp = att_stack.enter_context(tc.tile_pool(name="attp", bufs=1))
            Qt = [attp.tile([128, S], BF16, name=f"qt{i}") for i in range(2)]
            Kt = [attp.tile([128, S], BF16, name=f"kt{i}") for i in range(2)]
            Vaug = [attp.tile([128, 4, 65], BF16, name=f"va{kc}") for kc in range(16)]
            wop = att_stack.enter_context(tc.tile_pool(name="wop", bufs=1))
            wo_t = wop.tile([128, 2, D], BF16, name="wo_t")
            Ctx = [attp.tile([128, S], BF16, name=f"ctx{i}") for i in range(2)]

            # ------------- Phase P: transpose x + QKV projections -------------
            with ExitStack() as ph:
                psA = ph.enter_context(tc.tile_pool(name="psP", bufs=6, space="PSUM"))
                psT = ph.enter_context(tc.tile_pool(name="psT", bufs=2, space="PSUM"))
                xap = ph.enter_context(tc.tile_pool(name="xap", bufs=3))
                xtp = ph.enter_context(tc.tile_pool(name="xtp", bufs=1))
                wp = ph.enter_context(tc.tile_pool(name="wp", bufs=1))
                wq_t = wp.tile([128, 8, 256], BF16, name="wq_t")
                wk_t = wp.tile([128, 8, 256], BF16, name="wk_t")
                wv_t = wp.tile([128, 8, 256], BF16, name="wv_t")
                _xengs = cycle((nc.sync, nc.gpsimd, nc.scalar))

                def load_xa(ts):
                    xa = xap.tile([128, 4, D], BF16, name="xa")
                    for tc4 in range(4):
                        nstrip = 4 if ts == 0 else 2
                        w = 1024 // nstrip
                        for hh in range(nstrip):
                            next(_xengs).dma_start(
                                out=xa[:, tc4, hh * w:(hh + 1) * w],
                                in_=xg[ts * 512 + tc4 * 128:
                                       ts * 512 + (tc4 + 1) * 128,
                                       hh * w:(hh + 1) * w])
                    return xa

                xas = {0: load_xa(0)}
                _wengs = cycle((nc.gpsimd, nc.sync, nc.scalar))
                for wi, (_wt, _w) in enumerate(((wk_t, wk), (wq_t, wq), (wv_t, wv))):
                    for g4 in range(2):
                        next(_wengs).dma_start(
                            out=_wt[:, g4 * 4:(g4 + 1) * 4, :],
                            in_=_w[g4 * 512:(g4 + 1) * 512, :].rearrange(
                                "(a p) f -> p a f", p=128))
                    xas[wi + 1] = load_xa(wi + 1)
                for dc2 in range(2):
                    next(_wengs).dma_start(
                        out=wo_t[:, dc2, :], in_=wo[dc2 * 128:(dc2 + 1) * 128, :])
                # prefetch fp8 FFN weights for fg=0,1 (consumed in phase F)
                t["fw1"] = {0: load_fw(w1p, w1, 0, 1), 1: load_fw(w1p, w1, 1, 1)}
                t["fw2"] = {0: load_fw(w2p, w2, 0, 2), 1: load_fw(w2p, w2, 1, 2)}

                for ts in range(4):
                    xa = xas[ts]
                    Xts = [xtp.tile([128, 512], BF16, name=f"xt{dc}") for dc in range(8)]
                    for tc4 in range(4):
                        for dc in range(8):
                            pt = psT.tile([128, 128], BF16, name="ps")
                            nc.tensor.transpose(pt[:], xa[:, tc4, dc * 128:(dc + 1) * 128], ident[:])
                            nc.vector.tensor_copy(Xts[dc][:, tc4 * 128:(tc4 + 1) * 128], pt[:])
                    for w_t, b_sb, Dst in ((wk_t, bk_sb, Kt), (wq_t, bq_sb, Qt)):
                        for oc in range(2):
                            pk = psA.tile([128, 512], F32, name="ps")
                            for dc in range(8):
                                nc.tensor.matmul(pk[:], w_t[:, dc, oc * 128:(oc + 1) * 128],
                                                 Xts[dc][:], start=(dc == 0), stop=(dc == 7))
                            nc.vector.tensor_scalar_add(
                                out=Dst[oc][:, ts * 512:(ts + 1) * 512], in0=pk[:],
                                scalar1=b_sb[:, oc:oc + 1])
                    for tc4 in range(4):
                        kc = ts * 4 + tc4
                        pv = psA.tile([128, 256], F32, name="ps")
                        for dc in range(8):
                            nc.tensor.matmul(pv[:], Xts[dc][:, tc4 * 128:(tc4 + 1) * 128],
                                             wv_t[:, dc, :], start=(dc == 0), stop=(dc == 7))
                        nc.vector.tensor_tensor(
                            out=Vaug[kc][:, :, 0:64],
                            in0=pv[:].rearrange("p (h d) -> p h d", h=4),
                            in1=bv_b[:].rearrange("p (h d) -> p h d", h=4),
                            op=OP.add)
                        nc.vector.tensor_copy(Vaug[kc][:, :, 64:65], onesc[:])

            # ---- Phase A+O: attention interleaved with o-proj + chunked RS ----
            with ExitStack() as ph:
                expp = ph.enter_context(tc.tile_pool(name="expp", bufs=2))
                rzp = ph.enter_context(tc.tile_pool(name="rzp", bufs=2))
                stgp = ph.enter_context(tc.tile_pool(name="stgp", bufs=1))
                scP = ph.enter_context(tc.tile_pool(name="scP", bufs=2, space="PSUM"))
                psO = ph.enter_context(tc.tile_pool(name="psO", bufs=1, space="PSUM"))
                psB = ph.enter_context(tc.tile_pool(name="psB", bufs=2, space="PSUM"))
                psC = ph.enter_context(tc.tile_pool(name="psC", bufs=1, space="PSUM"))
                rawp = ph.enter_context(tc.tile_pool(name="rawp", bufs=2))
                for qb in range(4):
                    for hp in range(2):
                        avs = [psB.tile([65, 512], F32, name="av") for i in range(2)]
                        for kp in range(8):
                            for i in range(2):
                                sc = scP.tile([128, 1024], F32, name="sc2")
                                for half in range(2):
                                    kc = 2 * kp + half
                                    nc.tensor.matmul(
                                        sc[:, half * 512:(half + 1) * 512],
                                        Kt[hp][i * 64:(i + 1) * 64, kc * 128:(kc + 1) * 128],
                                        Qt[hp][i * 64:(i + 1) * 64, qb * 512:(qb + 1) * 512],
                                        start=True, stop=True)
                                e = expp.tile([128, 1024], BF16, name=f"e{i}")
                                nc.scalar.activation(e[:], sc[:], AF.Exp)
                                for half in range(2):
                                    kc = 2 * kp + half
                                    nc.tensor.matmul(
                                        avs[i][:], Vaug[kc][:, 2 * hp + i, :],
                                        e[:, half * 512:(half + 1) * 512],
                                        start=(kc == 0), stop=(kc == 15))
                        for i in range(2):
                            rz = rzp.tile([1, 512], F32R, name="rz")
                            with nc.allow_low_precision(reason="f32r is full width"):
                                nc.vector.reciprocal(rz[:], avs[i][64:65, :])
                            bcp = psC.tile([64, 512], F32, name="bcp")
                            nc.tensor.matmul(bcp[:], ones1[:], rz[:], start=True, stop=True)
                            rzs = rzp.tile([64, 512], F32, name="rzs")
                            nc.vector.tensor_copy(rzs[:], bcp[:])
                            nc.vector.tensor_mul(
                                out=Ctx[hp][i * 64:(i + 1) * 64, qb * 512:(qb + 1) * 512],
                                in0=avs[i][0:64, :], in1=rzs[:])
                    sA = stgp.tile([128, 4, D], BF16, name="sA")
                    for q4 in range(4):
                        tc16 = qb * 4 + q4
                        for oh in range(2):
                            po = psO.tile([128, 512], F32, name="po")
                            for dc2 in range(2):
                                nc.tensor.matmul(
                                    po[:], Ctx[dc2][:, tc16 * 128:(tc16 + 1) * 128],
                                    wo_t[:, dc2, oh * 512:(oh + 1) * 512],
                                    start=(dc2 == 0), stop=(dc2 == 1))
                            nc.vector.tensor_copy(sA[:, q4, oh * 512:(oh + 1) * 512], po[:])
                    for q4 in range(4):
                        (nc.sync, nc.gpsimd, nc.scalar, nc.sync)[q4].dma_start(
                            out=rs_in[(qb * 4 + q4) * 128:(qb * 4 + q4 + 1) * 128, :],
                            in_=sA[:, q4, :])
                    nc.gpsimd.collective_compute(
                        "ReduceScatter", OP.add,
                        ins=[rs_in[qb * 512:(qb + 1) * 512, :]],
                        outs=[rs_out[qb * 128:(qb + 1) * 128, :]],
                        replica_groups=GROUPS)
                    rawb = rawp.tile([128, D], BF16, name="rawb")
                    for hh in range(2):
                        (nc.sync, nc.gpsimd)[hh].dma_start(
                            out=rawb[:, hh * 512:(hh + 1) * 512],
                            in_=rs_out[qb * 128:(qb + 1) * 128,
                                       hh * 512:(hh + 1) * 512])
                    raw = rawp.tile([128, D], F32, name="raw")
                    nc.vector.tensor_add(out=raw[:], in0=rawb[:], in1=bo_b[:])
                    layernorm(A_t[qb], raw, ln1g_b, ln1b_b)

        # ------------- Phase F: FFN (fp8 DoubleRow) + LN2 -------------
        with ExitStack() as ph:
            sbA = ph.enter_context(tc.tile_pool(name="sbA", bufs=1))
            hp_ = ph.enter_context(tc.tile_pool(name="hp", bufs=2))
            fmisc = ph.enter_context(tc.tile_pool(name="fmisc", bufs=2))
            psA = ph.enter_context(tc.tile_pool(name="psF", bufs=4, space="PSUM"))
            psD = ph.enter_context(tc.tile_pool(name="psD", bufs=4, space="PSUM"))

            At = sbA.tile([128, 8, 512], F8 if FC1_FP8 else BF16, name="At")
            for tc4 in range(4):
                for dc in range(8):
                    pt = psA.tile([128, 128], BF16, name="ps")
                    nc.tensor.transpose(pt[:], A_t[tc4][:, dc * 128:(dc + 1) * 128], ident[:])
                    nc.scalar.copy(At[:, dc, tc4 * 128:(tc4 + 1) * 128], pt[:])

            ffn_acc = [sbA.tile([128, D], F32, name=f"fa{i}") for i in range(4)]
            oall = sbA.tile([128, 4, D], F32, name="oall")
            for tc4 in range(4):
                # ffn_acc = WSC*A + WSC*b2 (b2 pre-scaled on host); LN2 is
                # scale-invariant so the WSC factor cancels exactly there.
                eng = (nc.vector, nc.gpsimd)[tc4 % 2]
                if FC2_FP8:
                    eng.tensor_scalar(out=ffn_acc[tc4][:], in0=A_t[tc4][:],
                                      scalar1=WSC, scalar2=0.0,
                                      op0=OP.mult, op1=OP.add)
                    eng.tensor_tensor(out=ffn_acc[tc4][:], in0=ffn_acc[tc4][:],
                                      in1=b2_b[:], op=OP.add)
                else:
                    eng.tensor_tensor(out=ffn_acc[tc4][:], in0=A_t[tc4][:],
                                      in1=b2_b[:], op=OP.add)
            for fg in range(4):
                w1t = t["fw1"].pop(fg)
                w2t = t["fw2"].pop(fg)
                if fg + 2 < 4:
                    t["fw1"][fg + 2] = load_fw(w1p, w1, fg + 2, 1)
                    t["fw2"][fg + 2] = load_fw(w2p, w2, fg + 2, 2)
                hts = hp_.tile([128, 8, 512], F8 if FC2_FP8 else BF16, name="hts")
                for fc8 in range(8):
                    phm = psA.tile([128, 512], F32, name="ps")
                    if FC1_FP8:
                        for j in range(4):
                            nc.tensor.matmul(phm[:], w1t[:, 2 * j:2 * j + 2, fc8 * 128:(fc8 + 1) * 128],
                                             At[:, 2 * j:2 * j + 2, :],
                                             start=(j == 0), stop=(j == 3), perf_mode=DR)
                    else:
                        for dc in range(8):
                            nc.tensor.matmul(phm[:], w1t[:, dc, fc8 * 128:(fc8 + 1) * 128],
                                             At[:, dc, :], start=(dc == 0), stop=(dc == 7))
                    fci = fg * 8 + fc8
                    tmp = fmisc.tile([128, 512], F32, name="tmp")
                    nc.scalar.activation(out=tmp[:], in_=phm[:], func=AF.Relu,
                                         bias=b1_sb[:, fci:fci + 1],
                                         scale=(1.0 / WSC) if FC1_FP8 else 1.0)
                    nc.scalar.activation(hts[:, fc8, :], tmp[:], AF.Gelu)
                if fg < 3:
                    for oh in range(2):
                        paccs = [psD.tile([128, 512], F32, name="pac") for i in range(4)]
                        for tc4 in range(4):
                            if FC2_FP8:
                                for j in range(4):
                                    nc.tensor.matmul(paccs[tc4][:],
                                                     hts[:, 2 * j:2 * j + 2, tc4 * 128:(tc4 + 1) * 128],
                                                     w2t[:, 2 * j:2 * j + 2, oh * 512:(oh + 1) * 512],
                                                     start=(j == 0), stop=(j == 3), perf_mode=DR)
                            else:
                                for dc in range(8):
                                    nc.tensor.matmul(paccs[tc4][:],
                                                     hts[:, dc, tc4 * 128:(tc4 + 1) * 128],
                                                     w2t[:, dc, oh * 512:(oh + 1) * 512],
                                                     start=(dc == 0), stop=(dc == 7))
                        for tc4 in range(4):
                            dst = ffn_acc[tc4][:, oh * 512:(oh + 1) * 512]
                            nc.vector.tensor_add(out=dst, in0=dst, in1=paccs[tc4][:])
                else:
                    # last group tc4-major: LN2(tc4) overlaps fc2 of tc4+1
                    for tc4 in range(4):
                        paccs = [psD.tile([128, 512], F32, name="pac") for i in range(2)]
                        for oh in range(2):
                            if FC2_FP8:
                                for j in range(4):
                                    nc.tensor.matmul(paccs[oh][:],
                                                     hts[:, 2 * j:2 * j + 2, tc4 * 128:(tc4 + 1) * 128],
                                                     w2t[:, 2 * j:2 * j + 2, oh * 512:(oh + 1) * 512],
                                                     start=(j == 0), stop=(j == 3), perf_mode=DR)
                            else:
                                for dc in range(8):
                                    nc.tensor.matmul(paccs[oh][:],
                                                     hts[:, dc, tc4 * 128:(tc4 + 1) * 128],
                                                     w2t[:, dc, oh * 512:(oh + 1) * 512],
                                                     start=(dc == 0), stop=(dc == 7))
                        acc = ffn_acc[tc4]
                        for oh in range(2):
                            dst = acc[:, oh * 512:(oh + 1) * 512]
                            nc.vector.tensor_add(out=dst, in0=dst, in1=paccs[oh][:])
                        layernorm(oall[:, tc4, :], acc, ln2g_b, ln2b_b, split=True)
                        for s4 in range(4):
                            (nc.sync, nc.gpsimd, nc.scalar, nc.sync)[s4].dma_start(
                                out=out[tc4 * 128:(tc4 + 1) * 128,
                                        s4 * 256:(s4 + 1) * 256],
                                in_=oall[:, tc4, s4 * 256:(s4 + 1) * 256])


def _get_nc():
    if "nc" not in _CACHE:
        _CACHE["nc"] = _build()
    return _CACHE["nc"]


def _in_maps(inputs):
    x = np.asarray(inputs["x"], dtype=np.float32)
    F8NP = ml_dtypes.float8_e4m3
    BF = ml_dtypes.bfloat16
    if FC1_FP8:
        w1_c = (np.asarray(inputs["W1"], np.float32) * WSC).astype(F8NP)
    else:
        w1_c = np.asarray(inputs["W1"], np.float32).astype(BF)
    if FC2_FP8:
        w2_c = (np.asarray(inputs["W2"], np.float32) * WSC).astype(F8NP)
    else:
        w2_c = np.asarray(inputs["W2"], np.float32).astype(BF)
    maps = []
    for c in range(8):
        g, li = c // 4, c % 4
        cs = slice(256 * li, 256 * (li + 1))
        m = {
            "xg": np.ascontiguousarray(x[g]).astype(BF),
            "wq": (np.ascontiguousarray(np.asarray(inputs["Wq"], np.float32)[:, cs]) / 8.0).astype(BF),
            "wk": np.ascontiguousarray(np.asarray(inputs["Wk"], np.float32)[:, cs]).astype(BF),
            "wv": np.ascontiguousarray(np.asarray(inputs["Wv"], np.float32)[:, cs]).astype(BF),
            "wo": np.ascontiguousarray(np.asarray(inputs["Wo"], np.float32)[cs, :]).astype(BF),
            "w1": w1_c,
            "w2": w2_c,
            "bq": np.ascontiguousarray(np.asarray(inputs["bq"], np.float32)[cs]) / 8.0,
            "bk": np.ascontiguousarray(np.asarray(inputs["bk"], np.float32)[cs]),
            "bv": np.ascontiguousarray(np.asarray(inputs["bv"], np.float32)[cs]),
            "bo": np.asarray(inputs["bo"], np.float32),
            "b1": np.asarray(inputs["b1"], np.float32),
            "b2": np.asarray(inputs["b2"], np.float32) * (WSC if FC2_FP8 else 1.0),
            "ln1g": np.asarray(inputs["ln1_g"], np.float32),
            "ln1b": np.asarray(inputs["ln1_b"], np.float32),
            "ln2g": np.asarray(inputs["ln2_g"], np.float32),
            "ln2b": np.asarray(inputs["ln2_b"], np.float32),
        }
        maps.append(m)
    return maps


def run(inputs, trace=False):
    nc = _get_nc()
    res = run_bass_kernel_spmd(nc, _in_maps(inputs), list(range(8)), trace=trace)
    B = 2
    full = np.empty((B, S, D), np.float32)
    for c in range(8):
        g, li = c // 4, c % 4
        o = res.results[c]["out"]
        for j in range(4):
            full[g, j * 512 + li * 128: j * 512 + (li + 1) * 128, :] = \
                o[j * 128:(j + 1) * 128]
    return full, res


def kernel(**inputs):
    return run(inputs)[0]
